# revision 13
# baseline (speedup 1.0000x reference)
"""Causal self-attention (B=4, T=2048, D=1024, 16 heads) on 8 TRN2 NeuronCores.

Sharding: tensor-parallel over heads — each core owns 2 heads (a 128-dim slice
of the QKV projections, column-parallel) and the matching 128 rows of W_O
(row-parallel). Each core computes a full-shape partial output; the host sums
the 8 partials.

Per-core dataflow (all matmuls fp32r = full PE rate, ~tf32 accuracy):
  x.T [1024, 8192] (host-transposed, streamed in 512-token chunks)
  Q.T/K.T = W.T-slice @ x.T          -> [128, 2048] per batch (d-major)
  V       = x-chunk.T @ W_V.T-slice  -> [tok, 128] tiles with interleaved
            ones columns -> V' [128, 2*(64+1)] (ones column yields softmax
            sums for free in the PV matmul)
  S.T     = K-block @ Q.T-chunk      -> [128 keys, <=512 queries] per block,
            two heads row-packed in the 128-wide PE array (K=64 each)
  P.T     = exp(S.T / 8)  (ScalarE; no max-subtraction: scores ~ N(0,1))
            causal: above-diagonal blocks skipped, diagonal blocks masked by
            a constant upper-triangular 0/1 multiply
  out.T   = V'.T @ P.T accumulated over key blocks -> [65, 512] PSUM
            (row 64 = softmax sums)
  normalize: recip(sums) -> partition-broadcast -> multiply
  out_partial.T = W_O-slice.T-chunk @ attnout    -> [1024, 8192] streamed out

Projection matmuls for batch b+1 are interleaved between attention positions
of batch b (the attention inner loop is ACT-bound; dense interleaved PE work
keeps the tensor engine HAM-warm at 2.4 GHz).
"""
import os
import numpy as np
import concourse.bacc as bacc
import concourse.mybir as mybir
import concourse.tile as tile
from concourse import bass_utils

B, T, D = 4, 2048, 1024
NH, DH = 16, 64
NC = 8
HPC = NH // NC        # 2 heads per core
CS = HPC * DH         # 128 projection dims per core
TOK = B * T           # 8192 tokens
QC = 512              # query-chunk width
NCH = T // QC         # 4 chunks per batch
KT = D // 128         # 8 contraction tiles
NKB = T // 128        # 16 key blocks per batch
f32 = mybir.dt.float32
f32r = mybir.dt.float32r
AFT = mybir.ActivationFunctionType
SCALE = float(1.0 / np.sqrt(DH))
NV = DH + 1           # 65: V head columns + ones column

_cache = {}


def _build():
    if "nc" in _cache:
        return _cache["nc"]
    nc = bacc.Bacc("TRN2", target_bir_lowering=False, debug=False)

    xT_d = nc.dram_tensor("xT", [D, TOK], f32r, kind="ExternalInput").ap()
    WQT_d = nc.dram_tensor("WQT", [D, CS], f32r, kind="ExternalInput").ap()
    WKT_d = nc.dram_tensor("WKT", [D, CS], f32r, kind="ExternalInput").ap()
    WVT_d = nc.dram_tensor("WVT", [D, CS], f32r, kind="ExternalInput").ap()
    WOT_d = nc.dram_tensor("WOT", [CS, D], f32r, kind="ExternalInput").ap()
    umask_d = nc.dram_tensor("umask", [128, 128], f32r, kind="ExternalInput").ap()
    ones_d = nc.dram_tensor("onesc", [128, 2], f32r, kind="ExternalInput").ap()
    out_d = nc.dram_tensor("outT", [D, TOK], f32, kind="ExternalOutput").ap()

    with tile.TileContext(nc) as tc:
      with nc.allow_low_precision(reason="fp32r attention"):
        with tc.tile_pool(name="sb", bufs=1) as sb, \
             tc.tile_pool(name="sp", bufs=2) as sp, \
             tc.tile_pool(name="ps", bufs=1, space="PSUM") as ps:
            # ---- constants / weights (persistent)
            WQT_t = sb.tile([128, KT * CS], f32r, tag="wqt")
            WKT_t = sb.tile([128, KT * CS], f32r, tag="wkt")
            WVT_t = sb.tile([128, KT * CS], f32r, tag="wvt")
            for k in range(KT):
                nc.scalar.dma_start(out=WQT_t[:, k * CS:(k + 1) * CS], in_=WQT_d[k * 128:(k + 1) * 128, :])
                nc.scalar.dma_start(out=WKT_t[:, k * CS:(k + 1) * CS], in_=WKT_d[k * 128:(k + 1) * 128, :])
                nc.gpsimd.dma_start(out=WVT_t[:, k * CS:(k + 1) * CS], in_=WVT_d[k * 128:(k + 1) * 128, :])
            WOT_t = sb.tile([128, D], f32r, tag="wot")
            nc.gpsimd.dma_start(out=WOT_t[:], in_=WOT_d[:, :])
            umask_t = sb.tile([128, 128], f32r, tag="umask")
            nc.scalar.dma_start(out=umask_t[:], in_=umask_d[:, :])
            ones_t = sb.tile([128, 2], f32r, tag="ones")
            nc.scalar.dma_start(out=ones_t[:], in_=ones_d[:, :])

            qt = {}   # per-batch Q.T [128, T] fp32r
            kt = {}   # per-batch K.T [128, T]
            vp = {}   # (b, kb) -> V' [128, 2*NV]

            def proj_steps(b, ch):
                """QKV projection for chunk ch of batch b as emit-closures, so the
                matmuls can be interleaved between attention positions."""
                g = NCH * b + ch
                if ch == 0:
                    qt[b] = sp.tile([128, T], f32r, tag="qt", name=f"qt{b}", bufs=2)
                    kt[b] = sp.tile([128, T], f32r, tag="kt", name=f"kt{b}", bufs=2)
                xts = []

                def load_x():
                    for k in range(KT):
                        xtile = sp.tile([128, QC], f32r, tag=f"xt{k}", name=f"xt{k}_{g}", bufs=3)
                        nc.sync.dma_start(out=xtile[:], in_=xT_d[k * 128:(k + 1) * 128, g * QC:(g + 1) * QC])
                        xts.append(xtile)
                steps = [load_x]

                for wt, dst, nm in ((WQT_t, qt[b], "q"), (WKT_t, kt[b], "k")):
                    pp = ps.tile([128, QC], f32, tag="mm", name=f"pp{nm}{g}", bufs=2)
                    for k0 in range(0, KT, 2):
                        def fqk(wt=wt, dst=dst, k0=k0, pp=pp, ch=ch):
                            for k in (k0, k0 + 1):
                                nc.tensor.matmul(pp[:], wt[:, k * CS:(k + 1) * CS], xts[k][:],
                                                 start=(k == 0), stop=(k == KT - 1))
                            if k0 + 2 == KT:
                                nc.vector.tensor_copy(dst[:, ch * QC:(ch + 1) * QC], pp[:])
                        steps.append(fqk)

                for tt in range(QC // 128):
                    kb = ch * (QC // 128) + tt
                    vpt = sp.tile([128, 2 * NV], f32r, tag="vp", name=f"vp{b}_{kb}", bufs=2 * NKB)
                    vp[(b, kb)] = vpt
                    vpp = ps.tile([128, CS], f32, tag="mm", name=f"vpp{g}_{tt}", bufs=2)
                    for k0 in range(0, KT, 4):
                        def fv(tt=tt, k0=k0, vpt=vpt, vpp=vpp):
                            for k in range(k0, k0 + 4):
                                nc.tensor.matmul(vpp[:], xts[k][:, tt * 128:(tt + 1) * 128],
                                                 WVT_t[:, k * CS:(k + 1) * CS],
                                                 start=(k == 0), stop=(k == KT - 1))
                            if k0 + 4 == KT:
                                src2 = vpp[:].rearrange("p (h x) -> p h x", h=2)
                                dst2 = vpt[:].rearrange("p (h x) -> p h x", h=2)[:, :, 0:DH]
                                nc.vector.tensor_copy(dst2, src2)
                                nc.vector.tensor_copy(vpt[:].rearrange("p (h x) -> p h x", h=2)[:, :, DH:NV],
                                                      ones_t[:].rearrange("p (h x) -> p h x", h=2))
                        steps.append(fv)
                return steps

            pending = []  # queued proj closures, interleaved into attention

            def pull(n):
                for _ in range(min(n, len(pending))):
                    pending.pop(0)()

            def oproj_steps(g, ao):
                """O-projection for chunk g as filler steps (2 output tiles each)."""
                steps = []
                for mt in range(8):
                    def fo(mt=mt):
                        op = ps.tile([128, QC], f32, tag="mm", name=f"op{g}_{mt}", bufs=2)
                        nc.tensor.matmul(op[:], WOT_t[:, mt * 128:(mt + 1) * 128], ao[:],
                                         start=True, stop=True)
                        ot = sp.tile([128, QC], f32, tag="ot", name=f"ot{g}_{mt}", bufs=4)
                        nc.vector.tensor_copy(ot[:], op[:])
                        nc.sync.dma_start(out=out_d[mt * 128:(mt + 1) * 128, g * QC:(g + 1) * QC], in_=ot[:])
                    steps.append(fo)
                return steps

            def attn_chunk(b, ch, oproj_prev):
                """Attention + normalize for query chunk ch of batch b.
                Two-stage software pipeline: scores/exp for kb+1 are issued before
                the PV matmuls of kb, so the PV weight-loads never wait on exp.
                oproj_prev = (g, ao) of the previous chunk, interleaved here."""
                g = NCH * b + ch
                pvs = [ps.tile([65, QC], f32, tag=f"pv{h}", name=f"pv{h}_{g}", bufs=1) for h in range(HPC)]
                nkb = 4 * ch + 4

                def scores(kb):
                    off = max(0, 128 * kb - QC * ch)
                    sc = ps.tile([128, 2 * QC], f32, tag="sc", name=f"sc{g}_{kb}", bufs=2)
                    pt = sp.tile([128, 2 * QC], f32r, tag="pt", name=f"pt{g}_{kb}", bufs=4)
                    for h in range(HPC):
                        hb = h * QC
                        nc.tensor.matmul(sc[:, hb + off:hb + QC],
                                         kt[b][h * DH:(h + 1) * DH, kb * 128:(kb + 1) * 128],
                                         qt[b][h * DH:(h + 1) * DH, ch * QC + off:(ch + 1) * QC],
                                         start=True, stop=True)
                    if off == 0:
                        nc.scalar.activation(pt[:], sc[:], AFT.Exp, scale=SCALE)
                    else:
                        sc3 = sc[:].rearrange("p (h x) -> p h x", h=2)[:, :, off:QC]
                        pt3e = pt[:].rearrange("p (h x) -> p h x", h=2)[:, :, off:QC]
                        nc.scalar.activation(pt3e, sc3, AFT.Exp, scale=SCALE)
                    if 128 * kb >= QC * ch:  # diagonal block: mask keys > queries
                        pt3 = pt[:].rearrange("p (h x) -> p h x", h=2)[:, :, off:off + 128]
                        nc.vector.tensor_mul(pt3, pt3, umask_t[:].rearrange("p (o x) -> p o x", o=1).broadcast_to([128, 2, 128]))
                    return pt, off

                def pv_mm(kb, pt, off):
                    for h in range(HPC):
                        hb = h * QC
                        nc.tensor.matmul(pvs[h][:, off:QC],
                                         vp[(b, kb)][:, h * NV:(h + 1) * NV],
                                         pt[:, hb + off:hb + QC],
                                         start=(kb == 0), stop=(kb == nkb - 1))

                if oproj_prev is not None:
                    pending[0:0] = oproj_steps(*oproj_prev)
                q0 = scores(0)
                q1 = scores(1)
                for kb in range(2, nkb):
                    cur = scores(kb)
                    pull(2)
                    pv_mm(kb - 2, *q0)
                    q0, q1 = q1, cur
                pull(1)
                pv_mm(nkb - 2, *q0)
                pull(1)
                pv_mm(nkb - 1, *q1)
                # normalize -> attnout [128, 512] fp32r
                ao = sp.tile([128, QC], f32r, tag="ao", name=f"ao{g}", bufs=3)
                for h in range(HPC):
                    s_h = sp.tile([1, QC], f32, tag="sh", name=f"sh{g}_{h}", bufs=3)
                    r_h = sp.tile([1, QC], f32, tag="rh", name=f"rh{g}_{h}", bufs=3)
                    nc.vector.tensor_copy(s_h[0:1, :], pvs[h][64:65, :])
                    nc.vector.reciprocal_approx_fast(out=r_h[0:1, :], in_=s_h[0:1, :])
                    bc = sp.tile([DH, QC], f32, tag="bc", name=f"bc{g}_{h}", bufs=3)
                    nc.gpsimd.partition_broadcast(bc[:], r_h[0:1, :])
                    nc.vector.tensor_mul(ao[h * DH:(h + 1) * DH, :], pvs[h][0:DH, :], bc[:])
                return (g, ao)

            # emission: proj(0) fully, then attention with interleaved proj(b+1)
            for ch in range(NCH):
                for s in proj_steps(0, ch):
                    s()
            oprev = None
            for b in range(B):
                if b + 1 < B:
                    for ch in range(NCH):
                        pending.extend(proj_steps(b + 1, ch))
                for ch in range(NCH):
                    oprev = attn_chunk(b, ch, oprev)
                pull(len(pending))  # drain before next batch's attention
            for s in oproj_steps(*oprev):
                s()

    nc.compile()
    _cache["nc"] = nc
    return nc


def kernel(x, W_Q, W_K, W_V, W_O):
    nc = _build()
    xT = np.ascontiguousarray(np.asarray(x, dtype=np.float32).reshape(TOK, D).T)
    umask = np.triu(np.ones((128, 128), dtype=np.float32))
    onesc = np.ones((128, 2), dtype=np.float32)
    in_maps = []
    for c in range(NC):
        cs = slice(c * CS, (c + 1) * CS)
        in_maps.append({
            "xT": xT,
            "WQT": np.ascontiguousarray(np.asarray(W_Q, dtype=np.float32)[cs].T),
            "WKT": np.ascontiguousarray(np.asarray(W_K, dtype=np.float32)[cs].T),
            "WVT": np.ascontiguousarray(np.asarray(W_V, dtype=np.float32)[cs].T),
            "WOT": np.ascontiguousarray(np.asarray(W_O, dtype=np.float32)[:, cs].T),
            "umask": umask, "onesc": onesc,
        })
    trace = bool(os.environ.get("KERNEL_TRACE"))
    res = bass_utils.run_bass_kernel_spmd(nc, in_maps, list(range(NC)), trace=trace)
    kernel.last_result = res
    out = np.zeros((D, TOK), dtype=np.float64)
    for c in range(NC):
        out += res.results[c]["outT"].astype(np.float64)
    return np.ascontiguousarray(out.T.reshape(B, T, D)).astype(np.float32)
